# revision 2
# baseline (speedup 1.0000x reference)
"""GCN message-passing block on 8 Trainium2 NeuronCores.

Computes: delta = segment_sum((x @ W.T)[source] * edge_weights, target)

Strategy (edge-sharded, fully static SPMD program):
  By linearity, delta = segment_sum(x[source]*w, target) @ W.T -- the node
  projection commutes with the weighted aggregation, so W is applied AFTER
  aggregation (to ~100k rows) instead of per-edge (640k rows).

  Host side: each distinct target node gets a "compacted column". Columns
  are packed CPB=512 per PSUM bank; banks are distributed round-robin over
  the 8 cores. x is split into NCHUNK row-chunks so sources fit int16 for
  the hardware dma_gather. Within a bank, each (chunk c, stripe s) pair
  owns one gather tile of 128 slots; stripe s covers compact columns
  [64s, 64s+64). Edges overflowing their tile are deferred to later banks
  under fresh duplicate columns; the host adds duplicate rows at the end.

  Device side, per bank:
    1. NCHUNK dma_gathers fetch the source rows of x (512B each)
    2. DVE builds per-tile selectors S[e, col] = w_e * (tloc_e == col)
       via an iota-compare (batched over tiles)
    3. per tile: PE matmul Z[:, win] += X_tile.T @ S_tile accumulates the
       weighted segment sums for the bank's columns (dims on partitions)
    4. PE matmul out = Z_slice.T @ W.T flips orientation for free and
       applies the projection; result rows stream to DRAM contiguously.
"""

import numpy as np

import concourse.bacc as bacc
import concourse.bass as bass
import concourse.mybir as mybir
import concourse.tile as tile
from concourse.bass_utils import run_bass_kernel_spmd

N_CORES = 8
NUM_NODES = 100000
D = 128

NCHUNK = 4
CHUNK = NUM_NODES // NCHUNK   # 25000 rows per gather chunk (int16-addressable)
SWIDTH = 74      # columns per stripe == selector window width
NSTR = 7         # stripes per chunk (SWIDTH * NSTR >= CPB)
CPB = 512        # compacted columns per PSUM bank (one f32 bank)
SLOT = 128       # gather slots (edges) per tile
NB = 25          # banks per core
TPB = NCHUNK * NSTR            # tiles per bank (32)
SELBATCH = 8     # tiles per selector-build DVE op

NT = NB * TPB          # tiles per core
NCOL = NB * CPB        # output rows (compact columns) per core
NIDX = TPB * SLOT      # gather slots per bank (4096)
F32 = mybir.dt.float32
I16 = mybir.dt.int16


def _mk_ap(base, ap_list):
    return bass.AP(base.tensor, base.offset, ap_list)


def _bank_groups(nb, nbg):
    return [(g0, min(nbg, nb - g0)) for g0 in range(0, nb, nbg)]


def build_program(num_nodes=NUM_NODES, nb=NB, n_cores=N_CORES, stage_bufs=3,
                  repeat=1, do_gather=True, do_compute=True, n_queues=1,
                  single_packet=True, gsplit=1, nbg=1, psa_bufs=2,
                  psb_bufs=2, sel_bufs=3, zsb_bufs=2, osb_bufs=2):
    """Build + compile the single SPMD Bass program (data-independent).

    repeat>1 re-runs the whole pipeline (for slope-based benchmarking).
    """
    nt = nb * TPB
    ncol = nb * CPB
    chunk = num_nodes // NCHUNK
    nc = bacc.Bacc("TRN2", target_bir_lowering=False, debug=False,
                   num_devices=n_cores, num_swdge_queues=n_queues)
    x_t = nc.dram_tensor("x", [num_nodes, D], F32, kind="ExternalInput")
    wt_t = nc.dram_tensor("wt", [D, D], F32, kind="ExternalInput")
    # int16 gather indices: per (bank, chunk) a [128, SLOT*NSTR/16] block
    idx_t = nc.dram_tensor("idx16", [SLOT, nb * NCHUNK * (NSTR * SLOT // 16)],
                           I16, kind="ExternalInput")
    tloc_t = nc.dram_tensor("tloc", [SLOT, nt], F32, kind="ExternalInput")
    ew_t = nc.dram_tensor("ew", [SLOT, nt], F32, kind="ExternalInput")
    iota_t = nc.dram_tensor("iota", [SLOT, SWIDTH], F32, kind="ExternalInput")
    out_t = nc.dram_tensor("outc", [ncol, D], F32, kind="ExternalOutput")

    x_ap = x_t.ap()
    out_ap = out_t.ap()
    idxcols = NSTR * SLOT // 16   # 64 idx columns per (bank, chunk)

    with tile.TileContext(nc) as tc:
        with (
            tc.tile_pool(name="const", bufs=1) as constp,
            tc.tile_pool(name="stage", bufs=stage_bufs) as stagep,
            tc.tile_pool(name="sel", bufs=sel_bufs) as selp,
            tc.tile_pool(name="zsb", bufs=zsb_bufs) as zsbp,
            tc.tile_pool(name="outsb", bufs=osb_bufs) as outsbp,
            tc.tile_pool(name="psA", bufs=psa_bufs, space="PSUM") as psA,
            tc.tile_pool(name="psB", bufs=psb_bufs, space="PSUM") as psB,
        ):
            idx_sb = constp.tile([SLOT, nb * NCHUNK * idxcols], I16)
            tloc_sb = constp.tile([SLOT, nt], F32)
            ew_sb = constp.tile([SLOT, nt], F32)
            iota_sb = constp.tile([SLOT, SWIDTH], F32)
            wt_sb = constp.tile([D, D], F32)
            nc.sync.dma_start(idx_sb[:], idx_t.ap()[:])
            nc.sync.dma_start(tloc_sb[:], tloc_t.ap()[:])
            nc.sync.dma_start(ew_sb[:], ew_t.ap()[:])
            nc.sync.dma_start(iota_sb[:], iota_t.ap()[:])
            nc.sync.dma_start(wt_sb[:], wt_t.ap()[:])

            groups = _bank_groups(nb, nbg)
            gmax = max(gsz for _, gsz in groups)
            for _rep in range(repeat):
              iblk = 0   # cumulative idx16 column offset (in idxcols units)
              for b0, gsz in groups:
                # 1) gather: per chunk, one dma_gather covering gsz banks.
                # xg free layout: [chunk][bank-in-group][stripe][D]
                xg = stagep.tile([SLOT, gmax * TPB * D], F32, tag="xg")
                gidx = gsz * NSTR * SLOT           # idx per gather
                for c in range(NCHUNK if do_gather else 0):
                    g0 = c * gsz * NSTR
                    oslice = xg[:, g0 * D:(g0 + gsz * NSTR) * D]
                    o3 = oslice.rearrange("p (g e) -> p g e", e=D)
                    i0 = (iblk + c * gsz) * idxcols
                    nc.gpsimd.dma_gather(
                        out_ap=o3,
                        in_ap=x_ap[c * chunk:(c + 1) * chunk, :],
                        idxs_ap=idx_sb[:, i0:i0 + gidx // 16],
                        num_idxs=gidx,
                        num_idxs_reg=gidx,
                        elem_size=D,
                        queue_num=c % n_queues,
                        single_packet=single_packet,
                    )
                iblk += gsz * NCHUNK

                # 2) selector build: S[e, j, col] = ew * (tloc == col)
                if not do_compute:
                    continue
                for bg in range(gsz):
                  b = b0 + bg
                  t0 = b * TPB
                  sels = []
                  for g0 in range(0, TPB, SELBATCH):
                      gn = min(SELBATCH, TPB - g0)
                      S = selp.tile([SLOT, gn * SWIDTH], F32, tag="sel")
                      s3 = S[:].rearrange("p (g w) -> p g w", w=SWIDTH)
                      tl = tloc_sb[:, t0 + g0:t0 + g0 + gn]
                      tl_b = _mk_ap(tl, tl.ap[:2] + [[0, SWIDTH]])
                      io = iota_sb[:]
                      io_b = _mk_ap(io, io.ap[:1] + [[0, gn]] + io.ap[1:])
                      ew = ew_sb[:, t0 + g0:t0 + g0 + gn]
                      ew_b = _mk_ap(ew, ew.ap[:2] + [[0, SWIDTH]])
                      nc.vector.tensor_tensor(
                          out=s3, in0=tl_b, in1=io_b,
                          op=mybir.AluOpType.is_equal)
                      nc.vector.tensor_tensor(
                          out=s3, in0=s3, in1=ew_b,
                          op=mybir.AluOpType.mult)
                      sels.append((g0, S))

                  # 3) accumulate weighted segment sums into the PSUM bank
                  zp = psA.tile([SLOT, CPB], F32, tag="zp")
                  nc.vector.memset(zp[:], 0.0)
                  for j in range(TPB):
                      w0 = SWIDTH * (j % NSTR)
                      wd = min(SWIDTH, CPB - w0)
                      g0, S = sels[j // SELBATCH]
                      jj = j - g0
                      jc, js = j // NSTR, j % NSTR
                      xslice = (jc * gsz + bg) * NSTR + js
                      nc.tensor.matmul(
                          out=zp[:, w0:w0 + wd],
                          lhsT=xg[:, xslice * D:(xslice + 1) * D],
                          rhs=S[:, jj * SWIDTH:jj * SWIDTH + wd],
                          start=False, stop=(j == TPB - 1),
                          skip_group_check=True,
                      )

                  # 4) apply W.T: out rows (targets) = Z_slice.T @ W.T
                  zsb = zsbp.tile([SLOT, CPB], F32, tag="zsb")
                  nc.scalar.copy(zsb[:], zp[:])
                  ob = psB.tile([SLOT, CPB], F32, tag="ob")
                  for q in range(CPB // D):
                      nc.tensor.matmul(
                          out=ob[:, q * D:(q + 1) * D],
                          lhsT=zsb[:, q * D:(q + 1) * D],
                          rhs=wt_sb[:],
                          start=True, stop=True,
                      )
                  osb = outsbp.tile([SLOT, CPB], F32, tag="osb")
                  nc.scalar.copy(osb[:], ob[:])
                  dro = out_ap[b * CPB:(b + 1) * CPB, :].rearrange(
                      "(q p) d -> p q d", p=SLOT)
                  sro = osb[:].rearrange("p (q d) -> p q d", d=D)
                  nc.sync.dma_start(dro, sro)

    nc.compile()
    return nc


_PROGRAM_CACHE = {}

# tuned configuration (see bench history): 4 SWDGE queues so all four Q7
# core-pairs generate gather descriptors in parallel; multi-packet gathers;
# 6 staging buffers so many gathers stay in flight.
TUNED = dict(n_queues=4, single_packet=False, nbg=1, stage_bufs=6)


def _get_program(key="full", **kw):
    if key not in _PROGRAM_CACHE:
        _PROGRAM_CACHE[key] = build_program(**kw)
    return _PROGRAM_CACHE[key]


def preprocess(source, target, edge_weights, num_nodes=NUM_NODES, nb=NB,
               n_cores=N_CORES, nbg=1):
    """Assign edges to (core, bank, chunk, stripe, slot), targets to columns.

    Returns idx16 (replicated int16 gather indices), tloc, ew arrays, the
    column->target map, and leftover edges exceeding capacity (host handles;
    expected empty).
    """
    chunk = num_nodes // NCHUNK
    nt = nb * TPB
    n_banks = nb * n_cores
    idxcols = NSTR * SLOT // 16

    order = np.argsort(target, kind="stable")
    r_src = source[order].astype(np.int64)
    r_tgt = target[order].astype(np.int64)
    r_w = edge_weights[order].astype(np.float32)

    # idx stream per (core, bank, chunk): int16[NSTR*SLOT]; pad entries
    # spread across rows (same-row hammering serializes on one HBM row)
    pad = (np.arange(NSTR * SLOT, dtype=np.int64) * 97) % chunk
    idxs = np.broadcast_to(pad.astype(np.int16),
                           (n_cores, nb * NCHUNK, NSTR * SLOT)).copy()
    tloc = np.full((n_cores, SLOT, nt), -1.0, np.float32)
    ewa = np.zeros((n_cores, SLOT, nt), np.float32)
    colmap = np.full((n_cores, nb * CPB), -1, np.int64)

    gb = 0
    leftover = (np.zeros(0, np.int64), np.zeros(0, np.int64),
                np.zeros(0, np.float32))

    while r_tgt.size and gb < n_banks:
        ut, ucnt = np.unique(r_tgt, return_counts=True)
        n_u = ut.size
        ucol = 0
        ecur = 0
        defer = []
        while ucol < n_u and gb < n_banks:
            core = gb % n_cores
            bl = gb // n_cores
            take_u = min(CPB, n_u - ucol)
            bank_ut = ut[ucol:ucol + take_u]
            bank_cnt = ucnt[ucol:ucol + take_u]
            colmap[core, bl * CPB:bl * CPB + take_u] = bank_ut
            e_end = ecur + int(bank_cnt.sum())
            ecol = np.repeat(np.arange(take_u, dtype=np.int64), bank_cnt)
            b_src = r_src[ecur:e_end]
            b_tgt = r_tgt[ecur:e_end]
            b_w = r_w[ecur:e_end]
            b_chunk = b_src // chunk
            b_stripe = ecol // SWIDTH
            # order edges by (chunk, stripe) for grouped slot assignment
            o2 = np.lexsort((b_stripe, b_chunk))
            b_src, b_tgt, b_w = b_src[o2], b_tgt[o2], b_w[o2]
            ecol, b_chunk, b_stripe = ecol[o2], b_chunk[o2], b_stripe[o2]
            key = b_chunk * NSTR + b_stripe
            starts = np.searchsorted(key, np.arange(NCHUNK * NSTR + 1))
            for cs in range(NCHUNK * NSTR):
                lo, hi = int(starts[cs]), int(starts[cs + 1])
                n_e = hi - lo
                if n_e == 0:
                    continue
                c, s = cs // NSTR, cs % NSTR
                k = min(n_e, SLOT)
                sl = slice(lo, lo + k)
                ct = bl * TPB + c * NSTR + s          # tile index in core
                slots = np.arange(k)
                idxs[core, bl * NCHUNK + c, s * SLOT:s * SLOT + k] = (
                    b_src[sl] - c * chunk).astype(np.int16)
                tloc[core, slots, ct] = (ecol[sl] - SWIDTH * s
                                         ).astype(np.float32)
                ewa[core, slots, ct] = b_w[sl]
                if k < n_e:
                    dsl = slice(lo + k, hi)
                    defer.append((b_src[dsl], b_tgt[dsl], b_w[dsl]))
            ucol += take_u
            ecur = e_end
            gb += 1
        if ucol < n_u:
            defer.append((r_src[ecur:], r_tgt[ecur:], r_w[ecur:]))
        if defer:
            r_src = np.concatenate([d[0] for d in defer])
            r_tgt = np.concatenate([d[1] for d in defer])
            r_w = np.concatenate([d[2] for d in defer])
            o3 = np.argsort(r_tgt, kind="stable")
            r_src, r_tgt, r_w = r_src[o3], r_tgt[o3], r_w[o3]
        else:
            r_src = r_tgt = np.zeros(0, np.int64)
            r_w = np.zeros(0, np.float32)
    if r_tgt.size:
        leftover = (r_src, r_tgt, r_w)

    # regroup streams: one gather block per (bank-group, chunk); wrap into
    # the [128, .../16] int16 layout (pos i -> [i%16, i//16]), 8x replicated
    idx16 = np.zeros((n_cores, SLOT, nb * NCHUNK * idxcols), np.int16)
    col = 0
    strm = idxs.reshape(n_cores, nb, NCHUNK, NSTR * SLOT)
    for b0, gsz in _bank_groups(nb, nbg):
        for c in range(NCHUNK):
            blk = strm[:, b0:b0 + gsz, c, :].reshape(n_cores, -1)
            w = blk.shape[1] // 16
            st = blk.reshape(n_cores, w, 16).transpose(0, 2, 1)
            for k in range(8):
                idx16[:, 16 * k:16 * (k + 1), col:col + w] = st
            col += w
    return idx16, tloc, ewa, colmap, leftover


def _prep(x, W, edge_weights, src, tgt, n_cores=N_CORES):
    idx16, tloc, ewa, colmap, leftover = preprocess(
        src, tgt, edge_weights, nbg=TUNED["nbg"], n_cores=n_cores)
    wt = np.ascontiguousarray(W.T.astype(np.float32))
    iota = np.broadcast_to(np.arange(SWIDTH, dtype=np.float32),
                           (SLOT, SWIDTH)).copy()
    in_maps = [
        {"x": x, "wt": wt, "idx16": idx16[c], "tloc": tloc[c], "ew": ewa[c],
         "iota": iota}
        for c in range(n_cores)
    ]
    return in_maps, colmap, leftover


def make_in_maps(x, W, edge_weights, src, tgt, n_cores=N_CORES):
    return _prep(x, W, edge_weights, src, tgt, n_cores)[0]


def kernel(x, W, edge_weights, source, target):
    x = np.ascontiguousarray(np.asarray(x, np.float32))
    W = np.asarray(W, np.float32)
    edge_weights = np.asarray(edge_weights, np.float32)
    src = np.asarray(source).astype(np.int64)
    tgt = np.asarray(target).astype(np.int64)
    num_nodes, d = x.shape
    assert d == D and num_nodes == NUM_NODES, (x.shape,)

    in_maps, colmap, leftover = _prep(x, W, edge_weights, src, tgt)

    nc = _get_program("full", **TUNED)
    res = run_bass_kernel_spmd(nc, in_maps, core_ids=list(range(N_CORES)))

    out = np.zeros((num_nodes, D), np.float32)
    all_rows = np.concatenate([res.results[c]["outc"] for c in range(N_CORES)])
    all_cols = colmap.reshape(-1)
    valid = all_cols >= 0
    t_ids = all_cols[valid]
    rows = all_rows[valid]
    uniq, first = np.unique(t_ids, return_index=True)
    out[t_ids[first]] = rows[first]
    dup = np.ones(t_ids.size, bool)
    dup[first] = False
    if dup.any():
        np.add.at(out, t_ids[dup], rows[dup])
    l_src, l_tgt, l_w = leftover
    if l_tgt.size:
        np.add.at(out, l_tgt, (x[l_src] * l_w[:, None]) @ W.T)
    return out



# revision 15
# speedup vs baseline: 1089.5817x; 1089.5817x over previous
"""GCN message-passing block on 8 Trainium2 NeuronCores.

Computes: delta = segment_sum((x @ W.T)[source] * edge_weights, target)

Strategy (edge-sharded, fully static SPMD program):
  By linearity, delta = segment_sum(x[source]*w, target) @ W.T -- the node
  projection commutes with the weighted aggregation, so W is applied AFTER
  aggregation (to ~100k rows) instead of per-edge (640k rows).

  Host side: each distinct target node gets a "compacted column". Columns
  are packed CPB=512 per PSUM bank; banks are distributed round-robin over
  the 8 cores. x is split into NCHUNK row-chunks so sources fit int16 for
  the hardware dma_gather. Within a bank, each (chunk c, stripe s) pair
  owns one gather tile of 128 slots; stripe s covers compact columns
  [74s, 74s+74). Edges overflowing their tile are deferred to later banks
  under fresh duplicate columns; the host adds duplicate rows at the end.

  The whole device pipeline runs in bf16 (fp32 PSUM accumulation): x is
  pre-cast to bf16 on the host, halving gather DMA bytes (the dominant
  cost), quadrupling PE matmul throughput and doubling DVE selector-build
  throughput. The output is written bf16 and cast back to fp32 on the
  host. All rounding steps keep the relative error ~1e-3 << the 2e-2 gate.

  Device side, per bank group:
    1. NCHUNK dma_gathers fetch the source rows of x-bf16 (256B each)
    2. DVE builds per-tile selectors S[e, col] = w_e * (tloc_e == col)
       via an iota-compare (batched over tiles)
    3. per tile: PE matmul Z[:, win] += X_tile.T @ S_tile accumulates the
       weighted segment sums for the bank's columns (dims on partitions);
       the first tile of each column window writes with start=True so no
       PSUM memset is needed
    4. PE matmul out = Z_slice.T @ W.T flips orientation for free and
       applies the projection; result rows stream to DRAM contiguously.
"""

import numpy as np
import ml_dtypes

import concourse.bacc as bacc
import concourse.bass as bass
import concourse.mybir as mybir
import concourse.tile as tile
from concourse.bass_utils import run_bass_kernel_spmd

N_CORES = 8
NUM_NODES = 100000
D = 128

NCHUNK = 4
CHUNK = NUM_NODES // NCHUNK   # 25000 rows per gather chunk (int16-addressable)
SWIDTH = 74      # columns per stripe == selector window width
NSTR = 7         # stripes per chunk (SWIDTH * NSTR >= CPB)
CPB = 512        # compacted columns per PSUM bank (one f32 bank)
SLOT = 128       # gather slots (edges) per tile
NB = 25          # banks per core
TPB = NCHUNK * NSTR            # tiles per bank (28)
SELBATCH = 8     # tiles per selector-build DVE op
NT = NB * TPB          # tiles per core
NCOL = NB * CPB        # output rows (compact columns) per core
F32 = mybir.dt.float32
BF16 = mybir.dt.bfloat16
I16 = mybir.dt.int16
NP_BF16 = ml_dtypes.bfloat16

# dtype/structure config -- bisectable between fp32 baseline and bf16
CFG = dict(xdt=BF16, seldt=BF16, wdt=BF16, zdt=BF16, odt=BF16,
           use_memset=False, skip_mult=False, selbatch=28,
           skip_proj=False, skip_sel=False, sort_src=True,
           pair_gather=False, half_idx=False)


def np_of(dt):
    return {F32: np.float32, BF16: NP_BF16}[dt]


def _mk_ap(base, ap_list):
    return bass.AP(base.tensor, base.offset, ap_list)


def _bank_groups(nb, nbg):
    return [(g0, min(nbg, nb - g0)) for g0 in range(0, nb, nbg)]


def build_program(num_nodes=NUM_NODES, nb=NB, n_cores=N_CORES, stage_bufs=3,
                  repeat=1, do_gather=True, do_compute=True, n_queues=1,
                  single_packet=True, gsplit=1, nbg=1, psa_bufs=2,
                  psb_bufs=2, sel_bufs=3, zsb_bufs=2, osb_bufs=2):
    """Build + compile the single SPMD Bass program (data-independent).

    repeat>1 re-runs the whole pipeline (for slope-based benchmarking).
    """
    nt = nb * TPB
    ncol = nb * CPB
    chunk = num_nodes // NCHUNK
    nc = bacc.Bacc("TRN2", target_bir_lowering=False, debug=False,
                   num_devices=n_cores, num_swdge_queues=n_queues)
    XDT, SELDT, WDT = CFG["xdt"], CFG["seldt"], CFG["wdt"]
    ZDT, ODT = CFG["zdt"], CFG["odt"]
    x_t = nc.dram_tensor("x", [num_nodes, D], XDT, kind="ExternalInput")
    wt_t = nc.dram_tensor("wt", [D, D], WDT, kind="ExternalInput")
    # int16 gather indices: per (bank-group, chunk) a [128, gsz*SLOT*NSTR/16]
    # block
    idx_t = nc.dram_tensor("idx16", [SLOT, nb * NCHUNK * (NSTR * SLOT // 16)],
                           I16, kind="ExternalInput")
    tloc_t = nc.dram_tensor("tloc", [SLOT, nt], SELDT, kind="ExternalInput")
    ew_t = nc.dram_tensor("ew", [SLOT, nt], SELDT, kind="ExternalInput")
    SB = CFG["selbatch"]
    iota_t = nc.dram_tensor("iota", [SLOT, SWIDTH * SB], SELDT,
                            kind="ExternalInput")
    out_t = nc.dram_tensor("outc", [ncol, D], ODT, kind="ExternalOutput")

    x_ap = x_t.ap()
    out_ap = out_t.ap()
    idxcols = NSTR * SLOT // 16   # 56 idx columns per (bank, chunk)

    with tile.TileContext(nc) as tc:
        with (
            tc.tile_pool(name="const", bufs=1) as constp,
            tc.tile_pool(name="stage", bufs=stage_bufs) as stagep,
            tc.tile_pool(name="sel", bufs=sel_bufs) as selp,
            tc.tile_pool(name="zsb", bufs=zsb_bufs) as zsbp,
            tc.tile_pool(name="outsb", bufs=osb_bufs) as outsbp,
            tc.tile_pool(name="psA", bufs=psa_bufs, space="PSUM") as psA,
            tc.tile_pool(name="psB", bufs=psb_bufs, space="PSUM") as psB,
        ):
            idx_sb = constp.tile([SLOT, nb * NCHUNK * idxcols], I16)
            tloc_sb = constp.tile([SLOT, nt], SELDT)
            ew_sb = constp.tile([SLOT, nt], SELDT)
            iota_sb = constp.tile([SLOT, SWIDTH * SB], SELDT)
            wt_sb = constp.tile([D, D], WDT)
            nc.sync.dma_start(idx_sb[:], idx_t.ap()[:])
            nc.sync.dma_start(tloc_sb[:], tloc_t.ap()[:])
            nc.sync.dma_start(ew_sb[:], ew_t.ap()[:])
            nc.sync.dma_start(iota_sb[:], iota_t.ap()[:])
            nc.sync.dma_start(wt_sb[:], wt_t.ap()[:])

            groups = _bank_groups(nb, nbg)
            gmax = max(gsz for _, gsz in groups)
            for _rep in range(repeat):
              iblk = 0   # cumulative idx16 column offset (in idxcols units)
              for b0, gsz in groups:
                # 1) gather: per chunk, one dma_gather covering gsz banks.
                # xg free layout: [chunk][bank-in-group][stripe][D]
                PG = 2 if CFG["pair_gather"] else 1
                HI = 2 if CFG["half_idx"] else 1
                xg_tiles = TPB if HI == 1 else NCHUNK * 4
                xg = stagep.tile([SLOT, gmax * xg_tiles * D * PG], XDT,
                                 tag="xg")
                gidx = (gsz * NSTR * SLOT if HI == 1 else
                        gsz * 4 * SLOT)            # idx per gather
                for c in range(NCHUNK if do_gather else 0):
                    g0 = c * gsz * NSTR
                    nstr_eff = NSTR if HI == 1 else 4
                    g0e = c * gsz * nstr_eff
                    oslice = xg[:, g0e * D * PG:
                                (g0e + gsz * nstr_eff) * D * PG]
                    o3 = oslice.rearrange("p (g e) -> p g e", e=D * PG)
                    i0 = (iblk + c * gsz) * idxcols
                    nc.gpsimd.dma_gather(
                        out_ap=o3,
                        in_ap=(x_ap[c * chunk:(c + 1) * chunk, :]
                               if PG == 1 else
                               x_ap[c * chunk:(c + 1) * chunk, :].rearrange(
                                   "(a b) d -> a (b d)", b=PG)),
                        idxs_ap=idx_sb[:, i0:i0 + gidx // 16],
                        num_idxs=gidx,
                        num_idxs_reg=gidx,
                        elem_size=D * PG,
                        queue_num=c % n_queues,
                        single_packet=single_packet,
                    )
                iblk += gsz * NCHUNK

                # 2) selector build: S[e, j, col] = ew * (tloc == col)
                if not do_compute:
                    continue
                for bg in range(gsz):
                  b = b0 + bg
                  t0 = b * TPB
                  sels = []
                  for g0 in range(0, TPB, SB):
                      gn = min(SB, TPB - g0)
                      S = selp.tile([SLOT, gn * SWIDTH], SELDT, tag="sel")
                      s3 = S[:].rearrange("p (w g) -> p w g", g=gn)
                      tl = tloc_sb[:, t0 + g0:t0 + g0 + gn]
                      tl_b = _mk_ap(tl, tl.ap[:1] + [[0, SWIDTH]] + tl.ap[1:])
                      io_b = iota_sb[:, :SWIDTH * gn].rearrange(
                          "p (w g) -> p w g", g=gn)
                      ew = ew_sb[:, t0 + g0:t0 + g0 + gn]
                      ew_b = _mk_ap(ew, ew.ap[:1] + [[0, SWIDTH]] + ew.ap[1:])
                      if not CFG["skip_sel"]:
                          nc.vector.tensor_tensor(
                              out=s3, in0=tl_b, in1=io_b,
                              op=mybir.AluOpType.is_equal)
                          if not CFG["skip_mult"]:
                              nc.vector.tensor_tensor(
                                  out=s3, in0=s3, in1=ew_b,
                                  op=mybir.AluOpType.mult)
                      sels.append((g0, gn, S))

                  # 3) accumulate weighted segment sums into the PSUM bank;
                  # chunk 0 (j < NSTR) opens each column window with
                  # start=True so no memset is needed
                  zp = psA.tile([SLOT, CPB], F32, tag="zp")
                  if CFG["use_memset"]:
                      nc.vector.memset(zp[:], 0.0)
                  # stripe-major order: each column window's NCHUNK matmuls
                  # are consecutive, forming one PSUM accumulation group
                  # opened with start=True (no memset needed)
                  for js in range(NSTR):
                    w0 = SWIDTH * js
                    wd = min(SWIDTH, CPB - w0)
                    for jc in range(NCHUNK):
                      j = jc * NSTR + js
                      g0, gn, S = sels[j // SB]
                      jj = j - g0
                      Sj = S[:, jj:jj + 1]
                      rhs = _mk_ap(Sj, Sj.ap[:1] + [[gn, wd]])
                      xslice = (jc * gsz + bg) * NSTR + js
                      nc.tensor.matmul(
                          out=zp[:, w0:w0 + wd],
                          lhsT=xg[:, xslice * D:(xslice + 1) * D],
                          rhs=rhs,
                          start=(not CFG["use_memset"] and jc == 0),
                          stop=(jc == NCHUNK - 1),
                          skip_group_check=True,
                      )

                  # 4) apply W.T: out rows (targets) = Z_slice.T @ W.T
                  zsb = zsbp.tile([SLOT, CPB], ZDT, tag="zsb")
                  nc.scalar.copy(zsb[:], zp[:])
                  ob = psB.tile([SLOT, CPB], F32, tag="ob")
                  for q in range(0 if CFG["skip_proj"] else CPB // D):
                      nc.tensor.matmul(
                          out=ob[:, q * D:(q + 1) * D],
                          lhsT=zsb[:, q * D:(q + 1) * D],
                          rhs=wt_sb[:],
                          start=True, stop=True,
                      )
                  osb = outsbp.tile([SLOT, CPB], ODT, tag="osb")
                  nc.scalar.copy(osb[:], ob[:])
                  dro = out_ap[b * CPB:(b + 1) * CPB, :].rearrange(
                      "(q p) d -> p q d", p=SLOT)
                  sro = osb[:].rearrange("p (q d) -> p q d", d=D)
                  nc.sync.dma_start(dro, sro)

    nc.compile()
    return nc


_PROGRAM_CACHE = {}

# tuned configuration: 4 SWDGE queues so all four Q7 core-pairs generate
# gather descriptors in parallel; bank-pair gathers amortize the SWDGE
# fixed overhead; 6 staging buffers keep many gathers in flight.
TUNED = dict(n_queues=4, single_packet=False, nbg=1, stage_bufs=6,
             sel_bufs=8)


def _get_program(key="full", **kw):
    if key not in _PROGRAM_CACHE:
        _PROGRAM_CACHE[key] = build_program(**kw)
    return _PROGRAM_CACHE[key]


def preprocess(source, target, edge_weights, num_nodes=NUM_NODES, nb=NB,
               n_cores=N_CORES, nbg=1):
    """Assign edges to (core, bank, chunk, stripe, slot), targets to columns.

    Returns idx16 (replicated int16 gather indices), tloc, ew arrays, the
    column->target map, and leftover edges exceeding capacity (host handles;
    expected empty).
    """
    chunk = num_nodes // NCHUNK
    nt = nb * TPB
    n_banks = nb * n_cores
    idxcols = NSTR * SLOT // 16

    order = np.argsort(target, kind="stable")
    r_src = source[order].astype(np.int64)
    r_tgt = target[order].astype(np.int64)
    r_w = edge_weights[order].astype(np.float32)

    # idx stream per (core, bank, chunk): int16[NSTR*SLOT]; pad entries
    # spread across rows (same-row hammering serializes on one HBM row)
    pad = (np.arange(NSTR * SLOT, dtype=np.int64) * 97) % chunk
    idxs = np.broadcast_to(pad.astype(np.int16),
                           (n_cores, nb * NCHUNK, NSTR * SLOT)).copy()
    tloc = np.full((n_cores, SLOT, nt), -1.0, np.float32)
    ewa = np.zeros((n_cores, SLOT, nt), np.float32)
    colmap = np.full((n_cores, nb * CPB), -1, np.int64)

    gb = 0
    leftover = (np.zeros(0, np.int64), np.zeros(0, np.int64),
                np.zeros(0, np.float32))

    while r_tgt.size and gb < n_banks:
        ut, ucnt = np.unique(r_tgt, return_counts=True)
        n_u = ut.size
        ucol = 0
        ecur = 0
        defer = []
        while ucol < n_u and gb < n_banks:
            core = gb % n_cores
            bl = gb // n_cores
            take_u = min(CPB, n_u - ucol)
            bank_ut = ut[ucol:ucol + take_u]
            bank_cnt = ucnt[ucol:ucol + take_u]
            colmap[core, bl * CPB:bl * CPB + take_u] = bank_ut
            e_end = ecur + int(bank_cnt.sum())
            ecol = np.repeat(np.arange(take_u, dtype=np.int64), bank_cnt)
            b_src = r_src[ecur:e_end]
            b_tgt = r_tgt[ecur:e_end]
            b_w = r_w[ecur:e_end]
            b_chunk = b_src // chunk
            b_stripe = ecol // SWIDTH
            # order edges by (chunk, stripe) for grouped slot assignment
            o2 = np.lexsort((b_src, b_stripe, b_chunk)) \
                if CFG.get("sort_src") else np.lexsort((b_stripe, b_chunk))
            b_src, b_tgt, b_w = b_src[o2], b_tgt[o2], b_w[o2]
            ecol, b_chunk, b_stripe = ecol[o2], b_chunk[o2], b_stripe[o2]
            key = b_chunk * NSTR + b_stripe
            starts = np.searchsorted(key, np.arange(NCHUNK * NSTR + 1))
            for cs in range(NCHUNK * NSTR):
                lo, hi = int(starts[cs]), int(starts[cs + 1])
                n_e = hi - lo
                if n_e == 0:
                    continue
                c, s = cs // NSTR, cs % NSTR
                k = min(n_e, SLOT)
                sl = slice(lo, lo + k)
                ct = bl * TPB + c * NSTR + s          # tile index in core
                slots = np.arange(k)
                idxs[core, bl * NCHUNK + c, s * SLOT:s * SLOT + k] = (
                    b_src[sl] - c * chunk).astype(np.int16)
                tloc[core, slots, ct] = (ecol[sl] - SWIDTH * s
                                         ).astype(np.float32)
                ewa[core, slots, ct] = b_w[sl]
                if k < n_e:
                    dsl = slice(lo + k, hi)
                    defer.append((b_src[dsl], b_tgt[dsl], b_w[dsl]))
            ucol += take_u
            ecur = e_end
            gb += 1
        if ucol < n_u:
            defer.append((r_src[ecur:], r_tgt[ecur:], r_w[ecur:]))
        if defer:
            r_src = np.concatenate([d[0] for d in defer])
            r_tgt = np.concatenate([d[1] for d in defer])
            r_w = np.concatenate([d[2] for d in defer])
            o3 = np.argsort(r_tgt, kind="stable")
            r_src, r_tgt, r_w = r_src[o3], r_tgt[o3], r_w[o3]
        else:
            r_src = r_tgt = np.zeros(0, np.int64)
            r_w = np.zeros(0, np.float32)
    if r_tgt.size:
        leftover = (r_src, r_tgt, r_w)

    # regroup streams: one gather block per (bank-group, chunk); wrap into
    # the [128, .../16] int16 layout (pos i -> [i%16, i//16]), 8x replicated
    idx16 = np.zeros((n_cores, SLOT, nb * NCHUNK * idxcols), np.int16)
    col = 0
    strm = idxs.reshape(n_cores, nb, NCHUNK, NSTR * SLOT)
    for b0, gsz in _bank_groups(nb, nbg):
        for c in range(NCHUNK):
            blk = strm[:, b0:b0 + gsz, c, :].reshape(n_cores, -1)
            w = blk.shape[1] // 16
            st = blk.reshape(n_cores, w, 16).transpose(0, 2, 1)
            for k in range(8):
                idx16[:, 16 * k:16 * (k + 1), col:col + w] = st
            col += w
    return idx16, tloc, ewa, colmap, leftover


def _prep(x, W, edge_weights, src, tgt, n_cores=N_CORES):
    idx16, tloc, ewa, colmap, leftover = preprocess(
        src, tgt, edge_weights, nbg=TUNED["nbg"], n_cores=n_cores)
    xh = np.ascontiguousarray(x.astype(np_of(CFG["xdt"])))
    wt = np.ascontiguousarray(W.T.astype(np_of(CFG["wdt"])))
    sb = CFG["selbatch"]
    iota = np.broadcast_to(
        (np.arange(SWIDTH * sb) // sb).astype(np_of(CFG["seldt"])),
        (SLOT, SWIDTH * sb)).copy()
    in_maps = [
        {"x": xh, "wt": wt, "idx16": idx16[c],
         "tloc": tloc[c].astype(np_of(CFG["seldt"])),
         "ew": ewa[c].astype(np_of(CFG["seldt"])),
         "iota": iota}
        for c in range(n_cores)
    ]
    return in_maps, colmap, leftover


def make_in_maps(x, W, edge_weights, src, tgt, n_cores=N_CORES):
    return _prep(x, W, edge_weights, src, tgt, n_cores)[0]


def kernel(x, W, edge_weights, source, target):
    x = np.ascontiguousarray(np.asarray(x, np.float32))
    W = np.asarray(W, np.float32)
    edge_weights = np.asarray(edge_weights, np.float32)
    src = np.asarray(source).astype(np.int64)
    tgt = np.asarray(target).astype(np.int64)
    num_nodes, d = x.shape
    assert d == D and num_nodes == NUM_NODES, (x.shape,)

    in_maps, colmap, leftover = _prep(x, W, edge_weights, src, tgt)

    nc = _get_program("full", **TUNED)
    res = run_bass_kernel_spmd(nc, in_maps, core_ids=list(range(N_CORES)))

    out = np.zeros((num_nodes, D), np.float32)
    all_rows = np.concatenate(
        [res.results[c]["outc"].astype(np.float32) for c in range(N_CORES)])
    all_cols = colmap.reshape(-1)
    valid = all_cols >= 0
    t_ids = all_cols[valid]
    rows = all_rows[valid]
    uniq, first = np.unique(t_ids, return_index=True)
    out[t_ids[first]] = rows[first]
    dup = np.ones(t_ids.size, bool)
    dup[first] = False
    if dup.any():
        np.add.at(out, t_ids[dup], rows[dup])
    l_src, l_tgt, l_w = leftover
    if l_tgt.size:
        np.add.at(out, l_tgt, (x[l_src] * l_w[:, None]) @ W.T)
    return out
